# revision 2
# baseline (speedup 1.0000x reference)
"""flash_wave CA kernel for Trainium2 (Bass/Tile) - PSUM-accumulated shifts.

vs the baseline kernel.py: the 6-way input-channel reduction for output
channels 0..3 is folded into the PE shift matmuls via PSUM accumulation
(the shift matrix is the same for every input channel i, so
sum_i shift(D[o,i]*phi[i]) accumulates in PSUM across 6 matmuls per
range). This removes ~3.1us/step of DVE adds; DVE keeps only the 36
multiplies, the ch4/5 add tree, and the z-shift.

Clip is applied AFTER the shift (the reference's own order: clip(pn)) as
1-relu(1-x) using two Relu activation passes per PSUM bank on the Scalar
engine - all-Relu so the ACT function table is loaded once.
"""
import numpy as np

GRID = 32
CH = 6
RING = 16
T_CHUNK = 88

_build_cache = {}


def _build(T):
    if T in _build_cache:
        return _build_cache[T]
    import concourse.bacc as bacc
    import concourse.mybir as mybir
    from concourse.bass import AP
    from concourse.tile import TileContext

    F16 = mybir.dt.float16
    F32 = mybir.dt.float32
    OP = mybir.AluOpType
    AF = mybir.ActivationFunctionType

    nc = bacc.Bacc("TRN2", target_bir_lowering=False, debug=False)
    d_in = nc.dram_tensor("d_in", [128, CH * CH * 256], F16, kind="ExternalInput")
    phi0 = nc.dram_tensor("phi0", [128, CH * 256], F16, kind="ExternalInput")
    smat = nc.dram_tensor("smat", [128, 640], F16, kind="ExternalInput")
    frames = nc.dram_tensor("frames", [T, 128, CH * 256], F16, kind="ExternalOutput")

    D = nc.alloc_sbuf_tensor("D", [128, CH * CH * 256], F16)
    S = nc.alloc_sbuf_tensor("S", [128, 640], F16)
    ring = [nc.alloc_sbuf_tensor(f"ring{i}", [128, CH * 256], F16) for i in range(RING)]
    prod = nc.alloc_sbuf_tensor("prod", [128, CH * CH * 256], F16)
    t3 = nc.alloc_sbuf_tensor("t3", [128, 2 * 3 * 256], F16)
    u = nc.alloc_sbuf_tensor("u", [128, 2 * 256], F16)
    po = nc.alloc_sbuf_tensor("po", [128, CH * 256], F16)
    ta = nc.alloc_sbuf_tensor("ta", [128, 4 * 256], F16)
    ps0 = nc.alloc_psum_tensor("ps0", [128, 256], F32)
    ps1 = nc.alloc_psum_tensor("ps1", [128, 256], F32)
    ps2 = nc.alloc_psum_tensor("ps2", [128, 256], F32)
    ps3 = nc.alloc_psum_tensor("ps3", [128, 256], F32)

    with TileContext(nc):
        nc.sync.dma_start(D[:, :], d_in[:, :])
        nc.sync.dma_start(ring[RING - 1][:, :], phi0[:, :])
        nc.sync.dma_start(S[:, :], smat[:, :])

        D4 = D[:, :].rearrange("p (o i c) -> p o i c", o=CH, i=CH, c=256)
        prod4 = prod[:, :].rearrange("p (o i c) -> p o i c", o=CH, i=CH, c=256)

        def pe_stage(t, ii, first_i, last_i):
            """Shift+accumulate matmuls for input channels ii into ps0..ps3.
            Grouped by weight matrix so LDWEIGHTS happens once per group;
            each PSUM range gets start on its first matmul (stage 1) and
            stop on its last (stage 2)."""
            st = lambda i: first_i and i == ii[0]
            sp = lambda i: last_i and i == ii[-1]
            # identity: ch0 main (+x), ch1 main (-x)
            for i in ii:
                nc.tensor.matmul(ps0[:, 32:256], S[:, 512:640], prod4[:, 0, i, 0:224],
                                 start=st(i), stop=sp(i))
            for i in ii:
                nc.tensor.matmul(ps1[:, 0:224], S[:, 512:640], prod4[:, 1, i, 32:256],
                                 start=st(i), stop=sp(i))
            # x quadrant crossings
            for i in ii:
                nc.tensor.matmul(ps0[:, 0:32], S[:, 256:384], prod4[:, 0, i, 224:256],
                                 start=st(i), stop=sp(i))
            for i in ii:
                nc.tensor.matmul(ps1[:, 224:256], S[:, 384:512], prod4[:, 1, i, 0:32],
                                 start=st(i), stop=sp(i))
            # y shifts
            for i in ii:
                nc.tensor.matmul(ps2[:, :], S[:, 0:128], prod4[:, 2, i, :],
                                 start=st(i), stop=sp(i))
            for i in ii:
                nc.tensor.matmul(ps3[:, :], S[:, 128:256], prod4[:, 3, i, :],
                                 start=st(i), stop=sp(i))

        for t in range(T):
            prev = ring[(t + RING - 1) % RING]
            nxt = ring[t % RING]
            prev3 = prev[:, :].rearrange("p (i c) -> p i c", i=CH, c=256)
            phi_a = prev3[:, 4:6, :].unsqueeze(1).to_broadcast((128, CH, 2, 256))
            phi_c = prev3[:, 0:4, :].unsqueeze(1).to_broadcast((128, CH, 4, 256))

            # products for i in {4,5} first (their phi comes from DVE's own
            # z-shift writes of the previous step -> no ACT wait)
            nc.vector.tensor_tensor(prod4[:, :, 4:6, :], D4[:, :, 4:6, :], phi_a, op=OP.mult)
            pe_stage(t, [4, 5], first_i=True, last_i=False)

            nc.vector.tensor_tensor(prod4[:, :, 0:4, :], D4[:, :, 0:4, :], phi_c, op=OP.mult)
            pe_stage(t, [0, 1, 2, 3], first_i=False, last_i=True)

            # ch4/5 need po explicitly (z-shift is done on DVE)
            # a1: s[o,j,:] = prod[o,j,:] + prod[o,3+j,:]  for o in {4,5}
            nc.vector.tensor_tensor(
                AP(t3, 0, [[1536, 128], [768, 2], [1, 768]]),
                AP(prod, 4 * 1536, [[9216, 128], [1536, 2], [1, 768]]),
                AP(prod, 4 * 1536 + 768, [[9216, 128], [1536, 2], [1, 768]]),
                op=OP.add,
            )
            # a2: u[o,:] = s[o,0,:] + s[o,1,:]
            nc.vector.tensor_tensor(
                AP(u, 0, [[512, 128], [256, 2], [1, 256]]),
                AP(t3, 0, [[1536, 128], [768, 2], [1, 256]]),
                AP(t3, 256, [[1536, 128], [768, 2], [1, 256]]),
                op=OP.add,
            )
            # a3: po[o,:] = u[o,:] + s[o,2,:]   (o in {4,5})
            nc.vector.tensor_tensor(
                AP(po, 4 * 256, [[1536, 128], [256, 2], [1, 256]]),
                AP(u, 0, [[512, 128], [256, 2], [1, 256]]),
                AP(t3, 512, [[1536, 128], [768, 2], [1, 256]]),
                op=OP.add,
            )

            # PSUM -> nxt[0:4] with clip after shift: x -> relu(1-relu(1-x))
            for k, psk in enumerate((ps0, ps1, ps2, ps3)):
                nc.scalar.activation(ta[:, k * 256:(k + 1) * 256], psk[:, :],
                                     AF.Relu, bias=1.0, scale=-1.0)
                nc.scalar.activation(nxt[:, k * 256:(k + 1) * 256],
                                     ta[:, k * 256:(k + 1) * 256],
                                     AF.Relu, bias=1.0, scale=-1.0)

            # z shifts ch4/5 with clip (min into shifted position)
            zb = AP(nxt, 4 * 256, [[1536, 128], [287, 2], [32, 8]])
            nc.vector.memset(zb, 0.0)
            zout = AP(nxt, 4 * 256 + 1, [[1536, 128], [255, 2], [32, 8], [1, 31]])
            zin = AP(po, 4 * 256, [[1536, 128], [257, 2], [32, 8], [1, 31]])
            nc.vector.tensor_scalar_min(zout, zin, 1.0)

            nc.sync.dma_start(frames[t], nxt[:, :])
    nc.compile()
    _build_cache[T] = nc
    return nc


def _arrange_D(Dact):
    a = Dact.reshape(CH, CH, 4, 8, GRID, GRID)
    a = a.transpose(2, 4, 0, 1, 3, 5).reshape(128, CH * CH * 256)
    return np.ascontiguousarray(a).astype(np.float16)


def _arrange_state(phi):
    a = phi.reshape(CH, 4, 8, GRID, GRID).transpose(1, 3, 0, 2, 4).reshape(128, CH * 256)
    return np.ascontiguousarray(a).astype(np.float16)


def _unarrange_frames(fr):
    T = fr.shape[0]
    return (
        fr.reshape(T, 4, GRID, CH, 8, GRID)
        .transpose(0, 3, 1, 4, 2, 5)
        .reshape(T, CH, GRID, GRID, GRID)
        .astype(np.float32)
    )


def _make_smat():
    m = np.arange(128)
    S_up = ((m[None, :] - 1 == m[:, None]) & (m[None, :] % 32 != 0)).astype(np.float16)
    S_dn = ((m[None, :] + 1 == m[:, None]) & (m[None, :] % 32 != 31)).astype(np.float16)
    Sx_up = (m[:, None] == m[None, :] - 32).astype(np.float16)
    Sx_dn = (m[:, None] == m[None, :] + 32).astype(np.float16)
    I = np.eye(128, dtype=np.float16)
    return np.concatenate([S_up, S_dn, Sx_up, Sx_dn, I], axis=1)


def _run_chunk(nc, ins, retries=3):
    from concourse.bass_utils import run_bass_kernel_spmd

    last = None
    for _ in range(retries):
        try:
            res = run_bass_kernel_spmd(nc, [ins], core_ids=[0])
            return res.results[0]["frames"]
        except Exception as e:
            last = e
    raise last


def kernel(D, sx, sy, sz, ex, ey, ez, max_iterations):
    D = np.asarray(D, dtype=np.float32)
    sx, sy, sz = int(sx), int(sy), int(sz)
    ex, ey, ez = int(ex), int(ey), int(ez)
    T_total = int(max_iterations)

    phi0 = np.zeros((CH, GRID, GRID, GRID), np.float32)
    phi0[:, sx, sy, sz] = 1.0

    d_arr = _arrange_D(D + np.float32(0.95))
    smat = _make_smat()

    out = np.empty((T_total, CH, GRID, GRID, GRID), np.float32)
    out[0] = phi0

    state = phi0
    base = 0
    while base < T_total - 1:
        T = min(T_CHUNK, T_total - 1 - base)
        nc = _build(T)
        ins = {"d_in": d_arr, "phi0": _arrange_state(state), "smat": smat}
        fr = np.asarray(_run_chunk(nc, ins))
        frames = _unarrange_frames(fr)
        sums = frames[:, :, ex, ey, ez].sum(axis=1)
        hit = np.nonzero(sums > 0.01)[0]
        if hit.size:
            tstar_plus1 = base + 1 + int(hit[0])
            n_keep = min(tstar_plus1 - base, T)
            out[base + 1: base + 1 + n_keep] = frames[:n_keep]
            out[tstar_plus1 + 1:] = out[tstar_plus1]
            return out
        out[base + 1: base + 1 + T] = frames
        state = frames[T - 1]
        base += T
    return out


# revision 3
# speedup vs baseline: 1.0445x; 1.0445x over previous
"""flash_wave CA kernel for Trainium2 (Bass/Tile) - PSUM-accumulated shifts.

vs the baseline kernel.py: the 6-way input-channel reduction for output
channels 0..3 is folded into the PE shift matmuls via PSUM accumulation
(the shift matrix is the same for every input channel i, so
sum_i shift(D[o,i]*phi[i]) accumulates in PSUM across 6 matmuls per
range). This removes ~3.1us/step of DVE adds; DVE keeps only the 36
multiplies, the ch4/5 add tree, and the z-shift.

Clip is applied AFTER the shift (the reference's own order: clip(pn)) as
1-relu(1-x) using two Relu activation passes per PSUM bank on the Scalar
engine - all-Relu so the ACT function table is loaded once.
"""
import numpy as np

GRID = 32
CH = 6
RING = 16
T_CHUNK = 88

_build_cache = {}


def _build(T):
    if T in _build_cache:
        return _build_cache[T]
    import concourse.bacc as bacc
    import concourse.mybir as mybir
    from concourse.bass import AP
    from concourse.tile import TileContext

    F16 = mybir.dt.float16
    F32 = mybir.dt.float32
    OP = mybir.AluOpType
    AF = mybir.ActivationFunctionType

    nc = bacc.Bacc("TRN2", target_bir_lowering=False, debug=False)
    d_in = nc.dram_tensor("d_in", [128, CH * CH * 256], F16, kind="ExternalInput")
    phi0 = nc.dram_tensor("phi0", [128, CH * 256], F16, kind="ExternalInput")
    smat = nc.dram_tensor("smat", [128, 640], F16, kind="ExternalInput")
    frames = nc.dram_tensor("frames", [T, 128, CH * 256], F16, kind="ExternalOutput")

    D = nc.alloc_sbuf_tensor("D", [128, CH * CH * 256], F16)
    S = nc.alloc_sbuf_tensor("S", [128, 640], F16)
    ring = [nc.alloc_sbuf_tensor(f"ring{i}", [128, CH * 256], F16) for i in range(RING)]
    prod = nc.alloc_sbuf_tensor("prod", [128, CH * CH * 256], F16)
    t3 = nc.alloc_sbuf_tensor("t3", [128, 2 * 3 * 256], F16)
    u = nc.alloc_sbuf_tensor("u", [128, 2 * 256], F16)
    po = nc.alloc_sbuf_tensor("po", [128, CH * 256], F16)
    ta = nc.alloc_sbuf_tensor("ta", [128, 4 * 256], F16)
    ps0m = nc.alloc_psum_tensor("ps0m", [128, 224], F32)
    ps0c = nc.alloc_psum_tensor("ps0c", [128, 32], F32)
    ps1m = nc.alloc_psum_tensor("ps1m", [128, 224], F32)
    ps1c = nc.alloc_psum_tensor("ps1c", [128, 32], F32)
    ps2 = nc.alloc_psum_tensor("ps2", [128, 256], F32)
    ps3 = nc.alloc_psum_tensor("ps3", [128, 256], F32)

    with TileContext(nc):
        nc.sync.dma_start(D[:, :], d_in[:, :])
        nc.sync.dma_start(ring[RING - 1][:, :], phi0[:, :])
        nc.sync.dma_start(S[:, :], smat[:, :])

        D4 = D[:, :].rearrange("p (o i c) -> p o i c", o=CH, i=CH, c=256)
        prod4 = prod[:, :].rearrange("p (o i c) -> p o i c", o=CH, i=CH, c=256)

        def pe_stage(t, ii, first_i, last_i):
            """Shift+accumulate matmuls for input channels ii into ps0..ps3.
            Grouped by weight matrix so LDWEIGHTS happens once per group;
            each PSUM range gets start on its first matmul (stage 1) and
            stop on its last (stage 2)."""
            st = lambda i: first_i and i == ii[0]
            sp = lambda i: last_i and i == ii[-1]
            # identity: ch0 main (+x), ch1 main (-x)
            for i in ii:
                nc.tensor.matmul(ps0m[:, :], S[:, 512:640], prod4[:, 0, i, 0:224],
                                 start=st(i), stop=sp(i))
            for i in ii:
                nc.tensor.matmul(ps1m[:, :], S[:, 512:640], prod4[:, 1, i, 32:256],
                                 start=st(i), stop=sp(i))
            # x quadrant crossings (own banks: start=True resets a whole bank)
            for i in ii:
                nc.tensor.matmul(ps0c[:, :], S[:, 256:384], prod4[:, 0, i, 224:256],
                                 start=st(i), stop=sp(i))
            for i in ii:
                nc.tensor.matmul(ps1c[:, :], S[:, 384:512], prod4[:, 1, i, 0:32],
                                 start=st(i), stop=sp(i))
            # y shifts
            for i in ii:
                nc.tensor.matmul(ps2[:, :], S[:, 0:128], prod4[:, 2, i, :],
                                 start=st(i), stop=sp(i))
            for i in ii:
                nc.tensor.matmul(ps3[:, :], S[:, 128:256], prod4[:, 3, i, :],
                                 start=st(i), stop=sp(i))

        for t in range(T):
            prev = ring[(t + RING - 1) % RING]
            nxt = ring[t % RING]
            prev3 = prev[:, :].rearrange("p (i c) -> p i c", i=CH, c=256)
            phi_a = prev3[:, 4:6, :].unsqueeze(1).to_broadcast((128, CH, 2, 256))
            phi_c = prev3[:, 0:4, :].unsqueeze(1).to_broadcast((128, CH, 4, 256))

            # products for i in {4,5} first (their phi comes from DVE's own
            # z-shift writes of the previous step -> no ACT wait)
            nc.vector.tensor_tensor(prod4[:, :, 4:6, :], D4[:, :, 4:6, :], phi_a, op=OP.mult)
            pe_stage(t, [4, 5], first_i=True, last_i=False)

            nc.vector.tensor_tensor(prod4[:, :, 0:4, :], D4[:, :, 0:4, :], phi_c, op=OP.mult)
            pe_stage(t, [0, 1, 2, 3], first_i=False, last_i=True)

            # ch4/5 need po explicitly (z-shift is done on DVE)
            # a1: s[o,j,:] = prod[o,j,:] + prod[o,3+j,:]  for o in {4,5}
            nc.vector.tensor_tensor(
                AP(t3, 0, [[1536, 128], [768, 2], [1, 768]]),
                AP(prod, 4 * 1536, [[9216, 128], [1536, 2], [1, 768]]),
                AP(prod, 4 * 1536 + 768, [[9216, 128], [1536, 2], [1, 768]]),
                op=OP.add,
            )
            # a2: u[o,:] = s[o,0,:] + s[o,1,:]
            nc.vector.tensor_tensor(
                AP(u, 0, [[512, 128], [256, 2], [1, 256]]),
                AP(t3, 0, [[1536, 128], [768, 2], [1, 256]]),
                AP(t3, 256, [[1536, 128], [768, 2], [1, 256]]),
                op=OP.add,
            )
            # a3: po[o,:] = u[o,:] + s[o,2,:]   (o in {4,5})
            nc.vector.tensor_tensor(
                AP(po, 4 * 256, [[1536, 128], [256, 2], [1, 256]]),
                AP(u, 0, [[512, 128], [256, 2], [1, 256]]),
                AP(t3, 512, [[1536, 128], [768, 2], [1, 256]]),
                op=OP.add,
            )

            # PSUM -> nxt[0:4] with clip after shift: x -> relu(1-relu(1-x))
            segs = [(ps0c, 0, 32), (ps0m, 32, 256), (ps1m, 256, 480),
                    (ps1c, 480, 512), (ps2, 512, 768), (ps3, 768, 1024)]
            for psk, a, b in segs:
                nc.scalar.activation(ta[:, a:b], psk[:, :], AF.Relu, bias=1.0, scale=-1.0)
                nc.scalar.activation(nxt[:, a:b], ta[:, a:b], AF.Relu, bias=1.0, scale=-1.0)

            # z shifts ch4/5 with clip (min into shifted position)
            zb = AP(nxt, 4 * 256, [[1536, 128], [287, 2], [32, 8]])
            nc.vector.memset(zb, 0.0)
            zout = AP(nxt, 4 * 256 + 1, [[1536, 128], [255, 2], [32, 8], [1, 31]])
            zin = AP(po, 4 * 256, [[1536, 128], [257, 2], [32, 8], [1, 31]])
            nc.vector.tensor_scalar_min(zout, zin, 1.0)

            nc.sync.dma_start(frames[t], nxt[:, :])
    nc.compile()
    _build_cache[T] = nc
    return nc


def _arrange_D(Dact):
    a = Dact.reshape(CH, CH, 4, 8, GRID, GRID)
    a = a.transpose(2, 4, 0, 1, 3, 5).reshape(128, CH * CH * 256)
    return np.ascontiguousarray(a).astype(np.float16)


def _arrange_state(phi):
    a = phi.reshape(CH, 4, 8, GRID, GRID).transpose(1, 3, 0, 2, 4).reshape(128, CH * 256)
    return np.ascontiguousarray(a).astype(np.float16)


def _unarrange_frames(fr):
    T = fr.shape[0]
    return (
        fr.reshape(T, 4, GRID, CH, 8, GRID)
        .transpose(0, 3, 1, 4, 2, 5)
        .reshape(T, CH, GRID, GRID, GRID)
        .astype(np.float32)
    )


def _make_smat():
    m = np.arange(128)
    S_up = ((m[None, :] - 1 == m[:, None]) & (m[None, :] % 32 != 0)).astype(np.float16)
    S_dn = ((m[None, :] + 1 == m[:, None]) & (m[None, :] % 32 != 31)).astype(np.float16)
    Sx_up = (m[:, None] == m[None, :] - 32).astype(np.float16)
    Sx_dn = (m[:, None] == m[None, :] + 32).astype(np.float16)
    I = np.eye(128, dtype=np.float16)
    return np.concatenate([S_up, S_dn, Sx_up, Sx_dn, I], axis=1)


def _run_chunk(nc, ins, retries=3):
    from concourse.bass_utils import run_bass_kernel_spmd

    last = None
    for _ in range(retries):
        try:
            res = run_bass_kernel_spmd(nc, [ins], core_ids=[0])
            return res.results[0]["frames"]
        except Exception as e:
            last = e
    raise last


def kernel(D, sx, sy, sz, ex, ey, ez, max_iterations):
    D = np.asarray(D, dtype=np.float32)
    sx, sy, sz = int(sx), int(sy), int(sz)
    ex, ey, ez = int(ex), int(ey), int(ez)
    T_total = int(max_iterations)

    phi0 = np.zeros((CH, GRID, GRID, GRID), np.float32)
    phi0[:, sx, sy, sz] = 1.0

    d_arr = _arrange_D(D + np.float32(0.95))
    smat = _make_smat()

    out = np.empty((T_total, CH, GRID, GRID, GRID), np.float32)
    out[0] = phi0

    state = phi0
    base = 0
    while base < T_total - 1:
        T = min(T_CHUNK, T_total - 1 - base)
        nc = _build(T)
        ins = {"d_in": d_arr, "phi0": _arrange_state(state), "smat": smat}
        fr = np.asarray(_run_chunk(nc, ins))
        frames = _unarrange_frames(fr)
        sums = frames[:, :, ex, ey, ez].sum(axis=1)
        hit = np.nonzero(sums > 0.01)[0]
        if hit.size:
            tstar_plus1 = base + 1 + int(hit[0])
            n_keep = min(tstar_plus1 - base, T)
            out[base + 1: base + 1 + n_keep] = frames[:n_keep]
            out[tstar_plus1 + 1:] = out[tstar_plus1]
            return out
        out[base + 1: base + 1 + T] = frames
        state = frames[T - 1]
        base += T
    return out


# revision 4
# speedup vs baseline: 1.0486x; 1.0040x over previous
"""flash_wave CA kernel for Trainium2 (Bass/Tile) - PSUM-accumulated shifts.

vs the baseline kernel.py: the 6-way input-channel reduction for output
channels 0..3 is folded into the PE shift matmuls via PSUM accumulation
(the shift matrix is the same for every input channel i, so
sum_i shift(D[o,i]*phi[i]) accumulates in PSUM across 6 matmuls per
range). This removes ~3.1us/step of DVE adds; DVE keeps only the 36
multiplies, the ch4/5 add tree, and the z-shift.

Clip is applied AFTER the shift (the reference's own order: clip(pn)) as
1-relu(1-x) using two Relu activation passes per PSUM bank on the Scalar
engine - all-Relu so the ACT function table is loaded once.
"""
import numpy as np

GRID = 32
CH = 6
RING = 16
T_CHUNK = 88

_build_cache = {}


def _build(T):
    if T in _build_cache:
        return _build_cache[T]
    import concourse.bacc as bacc
    import concourse.mybir as mybir
    from concourse.bass import AP
    from concourse.tile import TileContext

    F16 = mybir.dt.float16
    F32 = mybir.dt.float32
    OP = mybir.AluOpType
    AF = mybir.ActivationFunctionType

    nc = bacc.Bacc("TRN2", target_bir_lowering=False, debug=False)
    d_in = nc.dram_tensor("d_in", [128, CH * CH * 256], F16, kind="ExternalInput")
    phi0 = nc.dram_tensor("phi0", [128, CH * 256], F16, kind="ExternalInput")
    smat = nc.dram_tensor("smat", [128, 640], F16, kind="ExternalInput")
    frames = nc.dram_tensor("frames", [T, 128, CH * 256], F16, kind="ExternalOutput")

    D = nc.alloc_sbuf_tensor("D", [128, CH * CH * 256], F16)
    S = nc.alloc_sbuf_tensor("S", [128, 640], F16)
    ring = [nc.alloc_sbuf_tensor(f"ring{i}", [128, CH * 256], F16) for i in range(RING)]
    prod = nc.alloc_sbuf_tensor("prod", [128, CH * CH * 256], F16)
    t3 = nc.alloc_sbuf_tensor("t3", [128, 2 * 3 * 256], F16)
    u = nc.alloc_sbuf_tensor("u", [128, 2 * 256], F16)
    po = nc.alloc_sbuf_tensor("po", [128, CH * 256], F16)
    ta = nc.alloc_sbuf_tensor("ta", [128, 4 * 256], F16)
    ps0m = nc.alloc_psum_tensor("ps0m", [128, 224], F32)
    ps0c = nc.alloc_psum_tensor("ps0c", [128, 32], F32)
    ps1m = nc.alloc_psum_tensor("ps1m", [128, 224], F32)
    ps1c = nc.alloc_psum_tensor("ps1c", [128, 32], F32)
    ps2 = nc.alloc_psum_tensor("ps2", [128, 256], F32)
    ps3 = nc.alloc_psum_tensor("ps3", [128, 256], F32)

    with TileContext(nc):
        nc.sync.dma_start(D[:, :], d_in[:, :])
        nc.sync.dma_start(ring[RING - 1][:, :], phi0[:, :])
        nc.sync.dma_start(S[:, :], smat[:, :])

        D4 = D[:, :].rearrange("p (o i c) -> p o i c", o=CH, i=CH, c=256)
        prod4 = prod[:, :].rearrange("p (o i c) -> p o i c", o=CH, i=CH, c=256)

        def pe_stage(t, ii, first_i, last_i):
            """Shift+accumulate matmuls for input channels ii into ps0..ps3.
            Grouped by weight matrix so LDWEIGHTS happens once per group;
            each PSUM range gets start on its first matmul (stage 1) and
            stop on its last (stage 2)."""
            st = lambda i: first_i and i == ii[0]
            sp = lambda i: last_i and i == ii[-1]
            # identity: ch0 main (+x), ch1 main (-x)
            for i in ii:
                nc.tensor.matmul(ps0m[:, :], S[:, 512:640], prod4[:, 0, i, 0:224],
                                 start=st(i), stop=sp(i))
            for i in ii:
                nc.tensor.matmul(ps1m[:, :], S[:, 512:640], prod4[:, 1, i, 32:256],
                                 start=st(i), stop=sp(i))
            # x quadrant crossings (own banks: start=True resets a whole bank)
            for i in ii:
                nc.tensor.matmul(ps0c[:, :], S[:, 256:384], prod4[:, 0, i, 224:256],
                                 start=st(i), stop=sp(i))
            for i in ii:
                nc.tensor.matmul(ps1c[:, :], S[:, 384:512], prod4[:, 1, i, 0:32],
                                 start=st(i), stop=sp(i))
            # y shifts
            for i in ii:
                nc.tensor.matmul(ps2[:, :], S[:, 0:128], prod4[:, 2, i, :],
                                 start=st(i), stop=sp(i))
            for i in ii:
                nc.tensor.matmul(ps3[:, :], S[:, 128:256], prod4[:, 3, i, :],
                                 start=st(i), stop=sp(i))

        for t in range(T):
            prev = ring[(t + RING - 1) % RING]
            nxt = ring[t % RING]
            prev3 = prev[:, :].rearrange("p (i c) -> p i c", i=CH, c=256)
            phi_a = prev3[:, 4:6, :].unsqueeze(1).to_broadcast((128, CH, 2, 256))
            phi_c = prev3[:, 0:4, :].unsqueeze(1).to_broadcast((128, CH, 4, 256))

            # products for i in {4,5} first (their phi comes from DVE's own
            # z-shift writes of the previous step -> no ACT wait)
            nc.vector.tensor_tensor(prod4[:, :, 4:6, :], D4[:, :, 4:6, :], phi_a, op=OP.mult)
            pe_stage(t, [4, 5], first_i=True, last_i=False)

            nc.vector.tensor_tensor(prod4[:, :, 0:4, :], D4[:, :, 0:4, :], phi_c, op=OP.mult)
            pe_stage(t, [0, 1, 2, 3], first_i=False, last_i=True)

            # ch4/5 need po explicitly (z-shift is done on DVE)
            # a1: s[o,j,:] = prod[o,j,:] + prod[o,3+j,:]  for o in {4,5}
            nc.vector.tensor_tensor(
                AP(t3, 0, [[1536, 128], [768, 2], [1, 768]]),
                AP(prod, 4 * 1536, [[9216, 128], [1536, 2], [1, 768]]),
                AP(prod, 4 * 1536 + 768, [[9216, 128], [1536, 2], [1, 768]]),
                op=OP.add,
            )
            # a2: u[o,:] = s[o,0,:] + s[o,1,:]
            nc.vector.tensor_tensor(
                AP(u, 0, [[512, 128], [256, 2], [1, 256]]),
                AP(t3, 0, [[1536, 128], [768, 2], [1, 256]]),
                AP(t3, 256, [[1536, 128], [768, 2], [1, 256]]),
                op=OP.add,
            )
            # a3: po[o,:] = u[o,:] + s[o,2,:]   (o in {4,5})
            nc.vector.tensor_tensor(
                AP(po, 4 * 256, [[1536, 128], [256, 2], [1, 256]]),
                AP(u, 0, [[512, 128], [256, 2], [1, 256]]),
                AP(t3, 512, [[1536, 128], [768, 2], [1, 256]]),
                op=OP.add,
            )

            # PSUM -> ta with clip part 1 on ACT: ta = relu(1 - ps) in [0,1]
            segs = [(ps0c, 0, 32), (ps0m, 32, 256), (ps1m, 256, 480),
                    (ps1c, 480, 512), (ps2, 512, 768), (ps3, 768, 1024)]
            for psk, a, b in segs:
                nc.scalar.activation(ta[:, a:b], psk[:, :], AF.Relu, bias=1.0, scale=-1.0)

            # z shifts ch4/5 with clip (min into shifted position)
            zb = AP(nxt, 4 * 256, [[1536, 128], [287, 2], [32, 8]])
            nc.vector.memset(zb, 0.0)
            zout = AP(nxt, 4 * 256 + 1, [[1536, 128], [255, 2], [32, 8], [1, 31]])
            zin = AP(po, 4 * 256, [[1536, 128], [257, 2], [32, 8], [1, 31]])
            nc.vector.tensor_scalar_min(zout, zin, 1.0)

            # clip part 2 on DVE: nxt[0:4] = 1 - ta  (= min(ps,1), one 4x op)
            nc.vector.tensor_scalar(nxt[:, 0:4 * 256], ta[:, :], -1.0, 1.0,
                                    op0=OP.mult, op1=OP.add)

            nc.sync.dma_start(frames[t], nxt[:, :])
    nc.compile()
    _build_cache[T] = nc
    return nc


def _arrange_D(Dact):
    a = Dact.reshape(CH, CH, 4, 8, GRID, GRID)
    a = a.transpose(2, 4, 0, 1, 3, 5).reshape(128, CH * CH * 256)
    return np.ascontiguousarray(a).astype(np.float16)


def _arrange_state(phi):
    a = phi.reshape(CH, 4, 8, GRID, GRID).transpose(1, 3, 0, 2, 4).reshape(128, CH * 256)
    return np.ascontiguousarray(a).astype(np.float16)


def _unarrange_frames(fr):
    T = fr.shape[0]
    return (
        fr.reshape(T, 4, GRID, CH, 8, GRID)
        .transpose(0, 3, 1, 4, 2, 5)
        .reshape(T, CH, GRID, GRID, GRID)
        .astype(np.float32)
    )


def _make_smat():
    m = np.arange(128)
    S_up = ((m[None, :] - 1 == m[:, None]) & (m[None, :] % 32 != 0)).astype(np.float16)
    S_dn = ((m[None, :] + 1 == m[:, None]) & (m[None, :] % 32 != 31)).astype(np.float16)
    Sx_up = (m[:, None] == m[None, :] - 32).astype(np.float16)
    Sx_dn = (m[:, None] == m[None, :] + 32).astype(np.float16)
    I = np.eye(128, dtype=np.float16)
    return np.concatenate([S_up, S_dn, Sx_up, Sx_dn, I], axis=1)


def _run_chunk(nc, ins, retries=3):
    from concourse.bass_utils import run_bass_kernel_spmd

    last = None
    for _ in range(retries):
        try:
            res = run_bass_kernel_spmd(nc, [ins], core_ids=[0])
            return res.results[0]["frames"]
        except Exception as e:
            last = e
    raise last


def kernel(D, sx, sy, sz, ex, ey, ez, max_iterations):
    D = np.asarray(D, dtype=np.float32)
    sx, sy, sz = int(sx), int(sy), int(sz)
    ex, ey, ez = int(ex), int(ey), int(ez)
    T_total = int(max_iterations)

    phi0 = np.zeros((CH, GRID, GRID, GRID), np.float32)
    phi0[:, sx, sy, sz] = 1.0

    d_arr = _arrange_D(D + np.float32(0.95))
    smat = _make_smat()

    out = np.empty((T_total, CH, GRID, GRID, GRID), np.float32)
    out[0] = phi0

    state = phi0
    base = 0
    while base < T_total - 1:
        T = min(T_CHUNK, T_total - 1 - base)
        nc = _build(T)
        ins = {"d_in": d_arr, "phi0": _arrange_state(state), "smat": smat}
        fr = np.asarray(_run_chunk(nc, ins))
        frames = _unarrange_frames(fr)
        sums = frames[:, :, ex, ey, ez].sum(axis=1)
        hit = np.nonzero(sums > 0.01)[0]
        if hit.size:
            tstar_plus1 = base + 1 + int(hit[0])
            n_keep = min(tstar_plus1 - base, T)
            out[base + 1: base + 1 + n_keep] = frames[:n_keep]
            out[tstar_plus1 + 1:] = out[tstar_plus1]
            return out
        out[base + 1: base + 1 + T] = frames
        state = frames[T - 1]
        base += T
    return out
